# revision 14
# baseline (speedup 1.0000x reference)
"""Trainium2 Bass kernel for a Direct-Form-II-transposed IIR single-step update.

reference semantics (all fp32):
    out  = input * b[0] + v[..., 0]                  # [B, C]
    v_new[..., m] = input * b[m+1] - out * a[m]      # m = 0..7
    v_new[..., m] += v[..., m+1]   for m < 7
returns (out [B, C], v_new [B, C, 8])

Sharding: data-parallel over dim 0 (batch) across 8 NeuronCores.
Per core N = 32*32768 = 1,048,576 rows. Rows are processed in tiles of
[128 partitions x F_t rows]; tile sizes taper at the head/tail so the
first compute starts (and the last store finishes) on a small chunk.

Per tile (engines balanced so DMA ~ DVE ~ 200us/core):
  ACT:  tm_m = b[m+1] * x                  (8 scale-copies, per-partition scalar)
  DVE:  out  = b[0]*x + v[:,0]             (scalar_tensor_tensor)
        vn[:, m] = -a[m]*out + tm_m        (8 STT, strided column writes)
        vn[:, 0:7] += v[:, 1:8]            (one inner-unit-7 add; full DVE rate)
  DMA:  loads on the SP HWDGE ring, stores on the ACT HWDGE ring.
"""

from contextlib import ExitStack

import numpy as np

import concourse.bass as bass
import concourse.tile as tile
from concourse import bacc, mybir
from concourse.bass_utils import run_bass_kernel_spmd

NCORES = 8
B, C, M = 256, 32768, 8
BP = B // NCORES          # batch rows per core
N = BP * C                # rows per core
P = 128                   # SBUF partitions

# rows-per-partition per tile; sum must be N // P = 8192
F_LIST = [256, 768, 1024, 1024, 1024, 1024, 1024, 1024, 768, 256]
assert sum(F_LIST) == N // P

F32 = mybir.dt.float32

_cached = None


def _build():
    nc = bacc.Bacc(
        "TRN2",
        target_bir_lowering=False,
        debug=False,
        enable_asserts=False,
    )

    x_h = nc.dram_tensor("x", [1, N], F32, kind="ExternalInput")
    v_h = nc.dram_tensor("v", [1, N * M], F32, kind="ExternalInput")
    b_d = nc.dram_tensor("b", [1, M + 1], F32, kind="ExternalInput").ap()
    a_d = nc.dram_tensor("a", [1, M], F32, kind="ExternalInput").ap()
    o_h = nc.dram_tensor("o", [1, N], F32, kind="ExternalOutput")
    vn_h = nc.dram_tensor("vn", [1, N * M], F32, kind="ExternalOutput")

    mult = mybir.AluOpType.mult
    add = mybir.AluOpType.add
    Copy = mybir.ActivationFunctionType.Copy

    def row_ap(handle, off_rows, fcount, width):
        # [128, width*fcount] AP: partition p covers `fcount` rows of
        # `width` elems starting at flat row off_rows + p*fcount
        return bass.AP(
            handle,
            off_rows * width,
            [[fcount * width, P], [1, fcount * width]],
        )

    with tile.TileContext(nc) as tc, ExitStack() as ctx:
        cpool = ctx.enter_context(tc.tile_pool(name="coef", bufs=1))
        xpool = ctx.enter_context(tc.tile_pool(name="xall", bufs=1))
        vpool = ctx.enter_context(tc.tile_pool(name="vin", bufs=2))
        vnpool = ctx.enter_context(tc.tile_pool(name="vout", bufs=2))
        opool = ctx.enter_context(tc.tile_pool(name="oout", bufs=2))
        tmpool = ctx.enter_context(tc.tile_pool(name="ttmp", bufs=4))

        # --- coefficient prep (one-time) -------------------------------
        row = cpool.tile([1, 2 * M + 1], F32)
        nc.sync.dma_start(row[:, 0 : M + 1], b_d[:])
        nc.sync.dma_start(row[:, M + 1 : 2 * M + 1], a_d[:])
        rep = cpool.tile([P, 2 * M + 1], F32)
        nc.gpsimd.partition_broadcast(rep[:], row[:])
        na = cpool.tile([P, M], F32)  # -a, replicated per partition
        nc.vector.tensor_scalar_mul(na[:], rep[:, M + 1 : 2 * M + 1], -1.0)
        b0r = rep[:, 0:1]

        x_all = xpool.tile([P, N // P], F32)

        # --- main loop -------------------------------------------------
        off = 0
        for F in F_LIST:
            xt = x_all[:, off : off + F]
            nc.sync.dma_start(xt, row_ap(x_h, off * P, F, 1))
            vt = vpool.tile([P, F * M], F32, tag="vt")
            nc.sync.dma_start(vt[:], row_ap(v_h, off * P, F, M))

            v3 = vt[:].rearrange("p (f m) -> p f m", m=M)
            vnt = vnpool.tile([P, F * M], F32, tag="vnt")
            vn3 = vnt[:].rearrange("p (f m) -> p f m", m=M)
            ot = opool.tile([P, F], F32, tag="ot")

            # out = x*b0 + v0
            nc.vector.scalar_tensor_tensor(ot[:], xt, b0r, v3[:, :, 0], mult, add)
            for m in range(M):
                tm = tmpool.tile([P, F], F32, tag="tm")
                nc.scalar.activation(
                    tm[:], xt, Copy, bias=0.0, scale=rep[:, m + 1 : m + 2]
                )
                nc.vector.scalar_tensor_tensor(
                    vn3[:, :, m], ot[:], na[:, m : m + 1], tm[:], mult, add
                )
            # vn[:, :, 0:7] += v[:, :, 1:8] — inner-unit AP, full DVE rate
            nc.vector.tensor_add(
                vn3[:, :, 0 : M - 1], vn3[:, :, 0 : M - 1], v3[:, :, 1:M]
            )

            nc.scalar.dma_start(row_ap(o_h, off * P, F, 1), ot[:])
            nc.scalar.dma_start(row_ap(vn_h, off * P, F, M), vnt[:])
            off += F

    nc.finalize()
    return nc


def _get_nc():
    global _cached
    if _cached is None:
        _cached = _build()
    return _cached


def _run(input, v, b, a, trace=False, **spmd_kwargs):
    nc = _get_nc()

    x = np.ascontiguousarray(np.asarray(input, dtype=np.float32)).reshape(B, C)
    vv = np.ascontiguousarray(np.asarray(v, dtype=np.float32))
    bb = np.ascontiguousarray(np.asarray(b, dtype=np.float32)).reshape(1, M + 1)
    aa = np.ascontiguousarray(np.asarray(a, dtype=np.float32)).reshape(1, M)

    in_maps = []
    for c in range(NCORES):
        xs = x[c * BP : (c + 1) * BP].reshape(1, N)
        vs = vv[c * BP : (c + 1) * BP].reshape(1, N * M)
        in_maps.append({"x": xs, "v": vs, "b": bb, "a": aa})

    res = run_bass_kernel_spmd(
        nc, in_maps, list(range(NCORES)), trace=trace, **spmd_kwargs
    )

    out = np.empty((B, C), dtype=np.float32)
    v_new = np.empty((B, C, M), dtype=np.float32)
    for c in range(NCORES):
        out[c * BP : (c + 1) * BP] = res.results[c]["o"].reshape(BP, C)
        v_new[c * BP : (c + 1) * BP] = res.results[c]["vn"].reshape(BP, C, M)
    return (out, v_new), res


def kernel(input, v, b, a):
    (out, v_new), _ = _run(input, v, b, a)
    return out, v_new


# revision 17
# speedup vs baseline: 1.0019x; 1.0019x over previous
"""Trainium2 Bass kernel for a Direct-Form-II-transposed IIR single-step update.

reference semantics (all fp32):
    out  = input * b[0] + v[..., 0]                  # [B, C]
    v_new[..., m] = input * b[m+1] - out * a[m]      # m = 0..7
    v_new[..., m] += v[..., m+1]   for m < 7
returns (out [B, C], v_new [B, C, 8])

Sharding: data-parallel over dim 0 (batch) across 8 NeuronCores.
Per core N = 32*32768 = 1,048,576 rows. Rows are processed in tiles of
[128 partitions x F_t rows]; tile sizes taper at the head/tail so the
first compute starts (and the last store finishes) on a small chunk.

Per tile (engines balanced so DMA ~ DVE ~ 200us/core):
  ACT:  tm_m = b[m+1] * x                  (8 scale-copies, per-partition scalar)
  DVE:  out  = b[0]*x + v[:,0]             (scalar_tensor_tensor)
        vn[:, m] = -a[m]*out + tm_m        (8 STT, strided column writes)
        vn[:, 0:7] += v[:, 1:8]            (one inner-unit-7 add; full DVE rate)
  DMA:  loads on the SP HWDGE ring, stores on the ACT HWDGE ring.
"""

from contextlib import ExitStack

import numpy as np

import concourse.bass as bass
import concourse.tile as tile
from concourse import bacc, mybir
from concourse.bass_utils import run_bass_kernel_spmd

NCORES = 8
B, C, M = 256, 32768, 8
BP = B // NCORES          # batch rows per core
N = BP * C                # rows per core
P = 128                   # SBUF partitions

# rows-per-partition per tile; sum must be N // P = 8192
F_LIST = [512, 1024, 1024, 1024, 1024, 1024, 1024, 1024, 512]
assert sum(F_LIST) == N // P

F32 = mybir.dt.float32

_cached = None


def _build():
    nc = bacc.Bacc(
        "TRN2",
        target_bir_lowering=False,
        debug=False,
        enable_asserts=False,
    )

    x_h = nc.dram_tensor("x", [1, N], F32, kind="ExternalInput")
    v_h = nc.dram_tensor("v", [1, N * M], F32, kind="ExternalInput")
    b_d = nc.dram_tensor("b", [1, M + 1], F32, kind="ExternalInput").ap()
    a_d = nc.dram_tensor("a", [1, M], F32, kind="ExternalInput").ap()
    o_h = nc.dram_tensor("o", [1, N], F32, kind="ExternalOutput")
    vn_h = nc.dram_tensor("vn", [1, N * M], F32, kind="ExternalOutput")

    mult = mybir.AluOpType.mult
    add = mybir.AluOpType.add
    Copy = mybir.ActivationFunctionType.Copy

    def row_ap(handle, off_rows, fcount, width):
        # [128, width*fcount] AP: partition p covers `fcount` rows of
        # `width` elems starting at flat row off_rows + p*fcount
        return bass.AP(
            handle,
            off_rows * width,
            [[fcount * width, P], [1, fcount * width]],
        )

    with tile.TileContext(nc) as tc, ExitStack() as ctx:
        cpool = ctx.enter_context(tc.tile_pool(name="coef", bufs=1))
        xpool = ctx.enter_context(tc.tile_pool(name="xin", bufs=3))
        vpool = ctx.enter_context(tc.tile_pool(name="vin", bufs=3))
        vnpool = ctx.enter_context(tc.tile_pool(name="vout", bufs=2))
        opool = ctx.enter_context(tc.tile_pool(name="oout", bufs=2))
        tmpool = ctx.enter_context(tc.tile_pool(name="ttmp", bufs=2))

        # --- coefficient prep (one-time) -------------------------------
        row = cpool.tile([1, 2 * M + 1], F32)
        nc.sync.dma_start(row[:, 0 : M + 1], b_d[:])
        nc.sync.dma_start(row[:, M + 1 : 2 * M + 1], a_d[:])
        rep = cpool.tile([P, 2 * M + 1], F32)
        nc.gpsimd.partition_broadcast(rep[:], row[:])
        na = cpool.tile([P, M], F32)  # -a, replicated per partition
        nc.vector.tensor_scalar_mul(na[:], rep[:, M + 1 : 2 * M + 1], -1.0)
        b0r = rep[:, 0:1]

        # --- main loop -------------------------------------------------
        off = 0
        for F in F_LIST:
            xtile = xpool.tile([P, F], F32, tag="xt")
            xt = xtile[:]
            nc.sync.dma_start(xt, row_ap(x_h, off * P, F, 1))
            vt = vpool.tile([P, F * M], F32, tag="vt")
            nc.sync.dma_start(vt[:], row_ap(v_h, off * P, F, M))

            v3 = vt[:].rearrange("p (f m) -> p f m", m=M)
            vnt = vnpool.tile([P, F * M], F32, tag="vnt")
            vn3 = vnt[:].rearrange("p (f m) -> p f m", m=M)
            ot = opool.tile([P, F], F32, tag="ot")

            # out = x*b0 + v0
            nc.vector.scalar_tensor_tensor(ot[:], xt, b0r, v3[:, :, 0], mult, add)
            for m in range(M):
                tm = tmpool.tile([P, F], F32, tag="tm")
                nc.scalar.activation(
                    tm[:], xt, Copy, bias=0.0, scale=rep[:, m + 1 : m + 2]
                )
                nc.vector.scalar_tensor_tensor(
                    vn3[:, :, m], ot[:], na[:, m : m + 1], tm[:], mult, add
                )
            # vn[:, :, 0:7] += v[:, :, 1:8] — inner-unit AP, full DVE rate
            nc.vector.tensor_add(
                vn3[:, :, 0 : M - 1], vn3[:, :, 0 : M - 1], v3[:, :, 1:M]
            )

            nc.scalar.dma_start(row_ap(o_h, off * P, F, 1), ot[:])
            nc.scalar.dma_start(row_ap(vn_h, off * P, F, M), vnt[:])
            off += F

    nc.finalize()
    return nc


def _get_nc():
    global _cached
    if _cached is None:
        _cached = _build()
    return _cached


def _run(input, v, b, a, trace=False, **spmd_kwargs):
    nc = _get_nc()

    x = np.ascontiguousarray(np.asarray(input, dtype=np.float32)).reshape(B, C)
    vv = np.ascontiguousarray(np.asarray(v, dtype=np.float32))
    bb = np.ascontiguousarray(np.asarray(b, dtype=np.float32)).reshape(1, M + 1)
    aa = np.ascontiguousarray(np.asarray(a, dtype=np.float32)).reshape(1, M)

    in_maps = []
    for c in range(NCORES):
        xs = x[c * BP : (c + 1) * BP].reshape(1, N)
        vs = vv[c * BP : (c + 1) * BP].reshape(1, N * M)
        in_maps.append({"x": xs, "v": vs, "b": bb, "a": aa})

    res = run_bass_kernel_spmd(
        nc, in_maps, list(range(NCORES)), trace=trace, **spmd_kwargs
    )

    out = np.empty((B, C), dtype=np.float32)
    v_new = np.empty((B, C, M), dtype=np.float32)
    for c in range(NCORES):
        out[c * BP : (c + 1) * BP] = res.results[c]["o"].reshape(BP, C)
        v_new[c * BP : (c + 1) * BP] = res.results[c]["vn"].reshape(BP, C, M)
    return (out, v_new), res


def kernel(input, v, b, a):
    (out, v_new), _ = _run(input, v, b, a)
    return out, v_new


# revision 18
# speedup vs baseline: 1.0947x; 1.0926x over previous
"""Trainium2 Bass kernel for a Direct-Form-II-transposed IIR single-step update.

reference semantics (all fp32):
    out  = input * b[0] + v[..., 0]                  # [B, C]
    v_new[..., m] = input * b[m+1] - out * a[m]      # m = 0..7
    v_new[..., m] += v[..., m+1]   for m < 7
returns (out [B, C], v_new [B, C, 8])

Sharding: data-parallel over dim 0 (batch) across 8 NeuronCores.
Per core N = 32*32768 = 1,048,576 rows. Rows are processed in tiles of
[128 partitions x F_t rows]; tile sizes taper at the head/tail so the
first compute starts (and the last store finishes) on a small chunk.

Per tile (engines balanced so DMA ~ DVE ~ 200us/core):
  ACT:  tm_m = b[m+1] * x                  (8 scale-copies, per-partition scalar)
  DVE:  out  = b[0]*x + v[:,0]             (scalar_tensor_tensor)
        vn[:, m] = -a[m]*out + tm_m        (8 STT, strided column writes)
        vn[:, 0:7] += v[:, 1:8]            (one inner-unit-7 add; full DVE rate)
  DMA:  loads on the SP HWDGE ring, stores on the ACT HWDGE ring.
"""

from contextlib import ExitStack

import numpy as np

import concourse.bass as bass
import concourse.tile as tile
from concourse import bacc, mybir
from concourse.bass_utils import run_bass_kernel_spmd

NCORES = 8
B, C, M = 256, 32768, 8
BP = B // NCORES          # batch rows per core
N = BP * C                # rows per core
P = 128                   # SBUF partitions

# rows-per-partition per tile; sum must be N // P = 8192
F_LIST = [1024, 1024, 1024, 1024, 1024, 1024, 1024, 1024]
assert sum(F_LIST) == N // P

F32 = mybir.dt.float32

_cached = None


def _build():
    nc = bacc.Bacc(
        "TRN2",
        target_bir_lowering=False,
        debug=False,
        enable_asserts=False,
    )

    x_h = nc.dram_tensor("x", [1, N], F32, kind="ExternalInput")
    v_h = nc.dram_tensor("v", [1, N * M], F32, kind="ExternalInput")
    b_d = nc.dram_tensor("b", [1, M + 1], F32, kind="ExternalInput").ap()
    a_d = nc.dram_tensor("a", [1, M], F32, kind="ExternalInput").ap()
    o_h = nc.dram_tensor("o", [1, N], F32, kind="ExternalOutput")
    vn_h = nc.dram_tensor("vn", [1, N * M], F32, kind="ExternalOutput")

    mult = mybir.AluOpType.mult
    add = mybir.AluOpType.add
    Copy = mybir.ActivationFunctionType.Copy

    def row_ap(handle, off_rows, fcount, width):
        # [128, width*fcount] AP: partition p covers `fcount` rows of
        # `width` elems starting at flat row off_rows + p*fcount
        return bass.AP(
            handle,
            off_rows * width,
            [[fcount * width, P], [1, fcount * width]],
        )

    with tile.TileContext(nc) as tc, ExitStack() as ctx:
        cpool = ctx.enter_context(tc.tile_pool(name="coef", bufs=1))
        xpool = ctx.enter_context(tc.tile_pool(name="xin", bufs=3))
        vpool = ctx.enter_context(tc.tile_pool(name="vin", bufs=3))
        vnpool = ctx.enter_context(tc.tile_pool(name="vout", bufs=2))
        opool = ctx.enter_context(tc.tile_pool(name="oout", bufs=2))
        tmpool = ctx.enter_context(tc.tile_pool(name="ttmp", bufs=2))

        # --- coefficient prep (one-time) -------------------------------
        row = cpool.tile([1, 2 * M + 1], F32)
        nc.sync.dma_start(row[:, 0 : M + 1], b_d[:])
        nc.sync.dma_start(row[:, M + 1 : 2 * M + 1], a_d[:])
        rep = cpool.tile([P, 2 * M + 1], F32)
        nc.gpsimd.partition_broadcast(rep[:], row[:])
        na = cpool.tile([P, M], F32)  # -a, replicated per partition
        nc.vector.tensor_scalar_mul(na[:], rep[:, M + 1 : 2 * M + 1], -1.0)
        b0r = rep[:, 0:1]

        # --- main loop -------------------------------------------------
        off = 0
        for F in F_LIST:
            xtile = xpool.tile([P, F], F32, tag="xt")
            xt = xtile[:]
            nc.sync.dma_start(xt, row_ap(x_h, off * P, F, 1))
            vt = vpool.tile([P, F * M], F32, tag="vt")
            nc.sync.dma_start(vt[:], row_ap(v_h, off * P, F, M))

            v3 = vt[:].rearrange("p (f m) -> p f m", m=M)
            vnt = vnpool.tile([P, F * M], F32, tag="vnt")
            vn3 = vnt[:].rearrange("p (f m) -> p f m", m=M)
            ot = opool.tile([P, F], F32, tag="ot")

            # out = x*b0 + v0
            nc.vector.scalar_tensor_tensor(ot[:], xt, b0r, v3[:, :, 0], mult, add)
            for m in range(M):
                tm = tmpool.tile([P, F], F32, tag="tm")
                nc.scalar.activation(
                    tm[:], xt, Copy, bias=0.0, scale=rep[:, m + 1 : m + 2]
                )
                nc.vector.scalar_tensor_tensor(
                    vn3[:, :, m], ot[:], na[:, m : m + 1], tm[:], mult, add
                )
            # vn[:, :, 0:7] += v[:, :, 1:8] — inner-unit AP, full DVE rate
            nc.vector.tensor_add(
                vn3[:, :, 0 : M - 1], vn3[:, :, 0 : M - 1], v3[:, :, 1:M]
            )

            nc.scalar.dma_start(row_ap(o_h, off * P, F, 1), ot[:])
            nc.scalar.dma_start(row_ap(vn_h, off * P, F, M), vnt[:])
            off += F

    nc.finalize()
    return nc


def _get_nc():
    global _cached
    if _cached is None:
        _cached = _build()
    return _cached


def _run(input, v, b, a, trace=False, **spmd_kwargs):
    nc = _get_nc()

    x = np.ascontiguousarray(np.asarray(input, dtype=np.float32)).reshape(B, C)
    vv = np.ascontiguousarray(np.asarray(v, dtype=np.float32))
    bb = np.ascontiguousarray(np.asarray(b, dtype=np.float32)).reshape(1, M + 1)
    aa = np.ascontiguousarray(np.asarray(a, dtype=np.float32)).reshape(1, M)

    in_maps = []
    for c in range(NCORES):
        xs = x[c * BP : (c + 1) * BP].reshape(1, N)
        vs = vv[c * BP : (c + 1) * BP].reshape(1, N * M)
        in_maps.append({"x": xs, "v": vs, "b": bb, "a": aa})

    res = run_bass_kernel_spmd(
        nc, in_maps, list(range(NCORES)), trace=trace, **spmd_kwargs
    )

    out = np.empty((B, C), dtype=np.float32)
    v_new = np.empty((B, C, M), dtype=np.float32)
    for c in range(NCORES):
        out[c * BP : (c + 1) * BP] = res.results[c]["o"].reshape(BP, C)
        v_new[c * BP : (c + 1) * BP] = res.results[c]["vn"].reshape(BP, C, M)
    return (out, v_new), res


def kernel(input, v, b, a):
    (out, v_new), _ = _run(input, v, b, a)
    return out, v_new
